# revision 5
# baseline (speedup 1.0000x reference)
"""Trainium2 Bass kernel for the HPNET loss (confidence + depth + rotation).

Contract: kernel(**inputs) takes the FULL unsharded inputs and returns the
full output (a tuple of three f32 scalars), distributing work across 8
NeuronCores internally.

Sharding (hardcoded): data-parallel over 8 cores.
  - confidence/confidence_gt/weight: batch dim 256 -> 32 batches per core,
    flattened per core to [128, 16384], downcast to fp16 on host (the
    streamed tensors dominate HBM traffic; fp16 halves it and the loss
    tolerates it: measured rel err ~1e-5).
  - depth_and_rotation/ann_values/ann_flags: ROI dim 8192 -> 1024 per core,
    laid out as [128, 8 ROIs * 5] f32 (flags as f32 mask [128, 8]).

Rotation loss via the quaternion identity (no quat2mat matrices):
  tr(M(q)^T M(p)) = 4<q,p>^2 - |q|^2 |p|^2  for the (unnormalized)
  quaternion-to-matrix map M, and M(p) @ RY = M(p x r_y) where r_y is the
  y-axis half-turn quaternion, so p' = p x r_y is just a signed component
  permutation. With G = M(q_dr / |q_dr|) and P = M(q_ann):
    ||G - P||_F^2      = 3 + 3 sA^2 + 2 sA - 8 <q_dr, q_ann>^2  / sD
    ||G - P RY||_F^2   = 3 + 3 sA^2 + 2 sA - 8 <q_dr, q_ann'>^2 / sD
  (sD = |q_dr|^2, sA = |q_ann|^2), and min(n1, n2) = sqrt(base - 8*max/sD).

DMA queues: a -> sync (HWDGE), b -> gpsimd (SWDGE), w -> scalar (HWDGE,
all chunks pre-issued before the Act squares so DMA issue never queues
behind compute). Per-core partial sums [128, n_chunks + 2] are reduced
on host.
"""

import numpy as np

_NCORES = 8
_B = 256
_HW = 256 * 256
_N = 8192
_PB = _B // _NCORES            # batches per core
_F = _PB * _HW // 128          # 16384 free elems per partition
_CHUNKS = (4096, 4096, 4096, 2048, 1024, 512, 512)
assert sum(_CHUNKS) == _F
_NCH = len(_CHUNKS)
_R = _N // _NCORES // 128      # 8 ROIs per partition
_OUTC = _NCH + 2

_CACHE = {}


def _emit_roi(nc, roi, f32, dr, ann, msk, accs):
    import concourse.mybir as mybir
    Alu = mybir.AluOpType
    Act = mybir.ActivationFunctionType
    AxX = mybir.AxisListType.X

    drt = roi.tile([128, _R * 5], f32, tag="drt", name="drt")
    annt = roi.tile([128, _R * 5], f32, tag="annt", name="annt")
    mt = roi.tile([128, _R], f32, tag="mt", name="mt")
    nc.scalar.dma_start(out=drt[:], in_=dr[:])
    nc.scalar.dma_start(out=annt[:], in_=ann[:])
    nc.scalar.dma_start(out=mt[:], in_=msk[:])

    dr3 = drt.rearrange("p (r c) -> p r c", c=5)   # [128, R, 5]
    an3 = annt.rearrange("p (r c) -> p r c", c=5)
    qd = dr3[:, :, 1:5]                            # [128, R, 4]
    qa = an3[:, :, 1:5]

    # depth loss partials
    dd = roi.tile([128, _R], f32, tag="dd", name="dd")
    nc.vector.tensor_sub(dd[:], dr3[:, :, 0], an3[:, :, 0])
    dd2 = roi.tile([128, _R], f32, tag="dd2", name="dd2")
    nc.scalar.activation(dd2[:], dd[:], Act.Square)
    dscr = roi.tile([128, _R], f32, tag="dscr", name="dscr")
    nc.vector.scalar_tensor_tensor(
        out=dscr[:], in0=dd2[:], scalar=1.0, in1=mt[:],
        op0=Alu.mult, op1=Alu.mult,
        accum_out=accs[:, _NCH:_NCH + 1])

    # q_dr' = (q2, q3, -q0, -q1): <q_dr', q_ann> = <q_dr, q_ann x r_y>
    qd2 = roi.tile([128, _R, 4], f32, tag="qd2", name="qd2")
    nc.vector.tensor_copy(qd2[:, :, 0:2], qd[:, :, 2:4])
    nc.vector.tensor_scalar_mul(qd2[:, :, 2:4], qd[:, :, 0:2], -1.0)

    # stacked products -> one reduce for sD, sA, dot1, dot2
    prod = roi.tile([128, 4, _R, 4], f32, tag="prod", name="prod")
    nc.vector.tensor_mul(prod[:, 0], qd, qd)
    nc.vector.tensor_mul(prod[:, 1], qa, qa)
    nc.vector.tensor_mul(prod[:, 2], qd, qa)
    nc.vector.tensor_mul(prod[:, 3], qd2[:], qa)
    red = roi.tile([128, 4, _R], f32, tag="red", name="red")
    nc.vector.tensor_reduce(out=red[:], in_=prod[:], axis=AxX, op=Alu.add)

    rinv = roi.tile([128, _R], f32, tag="rinv", name="rinv")
    nc.vector.reciprocal(rinv[:], red[:, 0, :])
    dsq = roi.tile([128, 2, _R], f32, tag="dsq", name="dsq")
    nc.vector.tensor_mul(dsq[:], red[:, 2:4, :], red[:, 2:4, :])
    kmax = roi.tile([128, _R], f32, tag="kmax", name="kmax")
    nc.vector.tensor_tensor(kmax[:], dsq[:, 0, :], dsq[:, 1, :], op=Alu.max)
    k = roi.tile([128, _R], f32, tag="k", name="k")
    nc.vector.tensor_mul(k[:], kmax[:], rinv[:])

    # base' = 1.5 sA^2 + sA ; nmin^2 = 2*(base' - 4k) + 3
    sa2 = roi.tile([128, _R], f32, tag="sa2", name="sa2")
    nc.scalar.activation(sa2[:], red[:, 1, :], Act.Square)
    basep = roi.tile([128, _R], f32, tag="basep", name="basep")
    nc.vector.scalar_tensor_tensor(
        out=basep[:], in0=sa2[:], scalar=1.5, in1=red[:, 1, :],
        op0=Alu.mult, op1=Alu.add)
    mp = roi.tile([128, _R], f32, tag="mp", name="mp")
    nc.vector.scalar_tensor_tensor(
        out=mp[:], in0=k[:], scalar=-4.0, in1=basep[:],
        op0=Alu.mult, op1=Alu.add)
    b3 = roi.tile([128, 1], f32, tag="b3", name="b3")
    nc.gpsimd.memset(b3[:], 3.0)
    n = roi.tile([128, _R], f32, tag="n", name="n")
    nc.scalar.activation(n[:], mp[:], Act.Sqrt, bias=b3[:], scale=2.0)
    rscr = roi.tile([128, _R], f32, tag="rscr", name="rscr")
    nc.vector.scalar_tensor_tensor(
        out=rscr[:], in0=n[:], scalar=1.0, in1=mt[:],
        op0=Alu.mult, op1=Alu.mult,
        accum_out=accs[:, _NCH + 1:_NCH + 2])


def build_nc():
    import concourse.bacc as bacc
    import concourse.mybir as mybir
    import concourse.tile as tile

    f32 = mybir.dt.float32
    f16 = mybir.dt.float16
    Alu = mybir.AluOpType
    Act = mybir.ActivationFunctionType

    nc = bacc.Bacc("TRN2", target_bir_lowering=False, debug=False,
                   num_devices=_NCORES)

    a = nc.dram_tensor("a", [128, _F], f16, kind="ExternalInput")
    b = nc.dram_tensor("b", [128, _F], f16, kind="ExternalInput")
    w = nc.dram_tensor("w", [128, _F], f16, kind="ExternalInput")
    dr = nc.dram_tensor("dr", [128, _R * 5], f32, kind="ExternalInput")
    ann = nc.dram_tensor("ann", [128, _R * 5], f32, kind="ExternalInput")
    msk = nc.dram_tensor("msk", [128, _R], f32, kind="ExternalInput")
    out = nc.dram_tensor("out", [128, _OUTC], f32, kind="ExternalOutput")

    with tile.TileContext(nc) as tc:
        with tc.tile_pool(name="io", bufs=2) as io, \
                tc.tile_pool(name="iow", bufs=_NCH) as iow, \
                tc.tile_pool(name="wk", bufs=3) as wk, \
                tc.tile_pool(name="roi", bufs=1) as roi:

            accs = roi.tile([128, _OUTC], f32, tag="accs", name="accs")

            tiles = []
            off = 0
            for i, ch in enumerate(_CHUNKS):
                at = io.tile([128, ch], f16, tag="at", name="at")
                bt = io.tile([128, ch], f16, tag="bt", name="bt")
                wt = iow.tile([128, ch], f16, tag="wt", name="wt")
                tiles.append((at, bt, wt, slice(off, off + ch)))
                off += ch

            # a0/b0 first so the HBM stream starts immediately
            at, bt, _, sl = tiles[0]
            nc.sync.dma_start(out=at[:], in_=a[:, sl])
            nc.gpsimd.dma_start(out=bt[:], in_=b[:, sl])

            # ROI losses: tiny DMAs (scalar queue) + a short serial chain
            # that hides under the first big chunk transfers.
            _emit_roi(nc, roi, f32, dr, ann, msk, accs)

            # all w chunks pre-issued on the scalar queue so no DMA issue
            # ever waits behind an Act square
            for i in range(_NCH):
                _, _, wt, sl = tiles[i]
                nc.scalar.dma_start(out=wt[:], in_=w[:, sl])

            ds = [None] * _NCH

            def emit_sub(i):
                at, bt, _, _ = tiles[i]
                d = wk.tile([128, _CHUNKS[i]], f16, tag="d", name="d")
                nc.vector.tensor_sub(d[:], at[:], bt[:])
                nc.scalar.activation(d[:], d[:], Act.Square)
                ds[i] = d

            def emit_stt(i):
                _, _, wt, _ = tiles[i]
                d = ds[i]
                nc.vector.scalar_tensor_tensor(
                    out=d[:], in0=d[:], scalar=1.0, in1=wt[:],
                    op0=Alu.mult, op1=Alu.mult,
                    accum_out=accs[:, i:i + 1])

            for i in range(_NCH):
                if i + 1 < _NCH:
                    at, bt, _, sl = tiles[i + 1]
                    nc.sync.dma_start(out=at[:], in_=a[:, sl])
                    nc.gpsimd.dma_start(out=bt[:], in_=b[:, sl])
                emit_sub(i)
                if i > 0:
                    emit_stt(i - 1)
            emit_stt(_NCH - 1)

            nc.sync.dma_start(out=out[:], in_=accs[:])

    nc.compile()
    return nc


def _get_nc():
    if "nc" not in _CACHE:
        _CACHE["nc"] = build_nc()
    return _CACHE["nc"]


def make_in_maps(confidence, confidence_gt, weight, depth_and_rotation,
                 ann_values, ann_flags):
    a = np.ascontiguousarray(confidence, dtype=np.float16).reshape(
        _NCORES, 128, _F)
    b = np.ascontiguousarray(confidence_gt, dtype=np.float16).reshape(
        _NCORES, 128, _F)
    w = np.ascontiguousarray(weight, dtype=np.float16).reshape(
        _NCORES, 128, _F)
    dr = np.ascontiguousarray(depth_and_rotation, dtype=np.float32).reshape(
        _NCORES, 128, _R * 5)
    an = np.ascontiguousarray(ann_values, dtype=np.float32).reshape(
        _NCORES, 128, _R * 5)
    mk = np.ascontiguousarray(ann_flags).astype(np.float32).reshape(
        _NCORES, 128, _R)
    return [dict(a=a[c], b=b[c], w=w[c], dr=dr[c], ann=an[c], msk=mk[c])
            for c in range(_NCORES)]


def reduce_outs(outs):
    """outs: list of per-core {'out': [128, _OUTC]} -> (conf, depth, rot)."""
    P = np.stack([o["out"] for o in outs]).astype(np.float64)
    conf = P[:, :, :_NCH].sum() / float(_HW)
    dep = P[:, :, _NCH].sum() / float(_N)
    rot = P[:, :, _NCH + 1].sum() / float(_N)
    return (np.float32(conf), np.float32(dep), np.float32(rot))


def kernel(confidence, confidence_gt, weight, depth_and_rotation,
           ann_values, ann_flags):
    from concourse.bass_utils import run_bass_kernel_spmd
    nc = _get_nc()
    in_maps = make_in_maps(confidence, confidence_gt, weight,
                           depth_and_rotation, ann_values, ann_flags)
    res = run_bass_kernel_spmd(nc, in_maps, core_ids=list(range(_NCORES)))
    return reduce_outs(res.results)
